# revision 1
# baseline (speedup 1.0000x reference)
"""MixFFN MoE-routing kernel for Trainium2 (8 NeuronCores, token-parallel).

Math (per token block):
    logits = x @ gate_w.T ; probs = softmax(logits); top2 -> ew [N, E] (dense, rows sum to 1)
    CW1 = x @ W1.T ; CW3 = x @ W3.T
    per expert e:
        w1_e = CW1 + (x @ A1e.T) @ B1e.T
        w3_e = CW3 + (x @ A3e.T) @ B3e.T
        h_e  = silu(w1_e) * w3_e
    out = (sum_e ew_e * h_e) @ W2.T + sum_e ((ew_e * h_e) @ A2e.T) @ B2e.T

Key restructuring vs the reference scan: row-scaling by ew commutes with the
right matmuls, so the big W2 GEMM runs once on H = sum_e ew_e*h_e instead of
once per expert.  The u-projection (A2 contraction) is computed from the
unscaled h_e and column-scaled by ew afterwards (column scaling commutes with
contraction over dff).

Sharding: token-parallel.  Each of the 8 cores gets N/8 = 512 tokens and a
replicated copy of all weights; outputs are disjoint row blocks (no
collectives).  All layout transposes / dtype casts are done host-side.

On-chip layout: feature-on-partition ("transposed"), activations [feat, tok].
"""

import numpy as np

# problem dims (hardcoded per harness contract)
N, D, DFF, E, KTOP, R = 4096, 2048, 8192, 8, 2, 16
NCORES = 8
P = 128

_CACHE = {}


def build_bass(D_=D, DFF_=DFF, E_=E, R_=R, NTOK=N // NCORES, repeat=1):
    """Build the per-core Bass program (same SPMD program on every core)."""
    import concourse.bass as bass
    import concourse.mybir as mybir
    from concourse import bacc
    from concourse.tile import TileContext
    from concourse.masks import make_identity

    dt = mybir.dt
    op = mybir.AluOpType
    AF = mybir.ActivationFunctionType

    KD = D_ // P      # contraction tiles over D
    KF = DFF_ // P    # dff tiles
    MD = D_ // P      # output d tiles
    TT = NTOK // P    # token tiles
    ER = E_ * R_      # stacked expert-rank dim (=128 at full size)

    nc = bacc.Bacc("TRN2", target_bir_lowering=False, debug=False)

    # ---- DRAM I/O ----
    # all inputs pre-swizzled host-side so every DMA is 128 contiguous
    # descriptors (partition-major tiles), not thousands of 256B chunks
    x_bf = nc.dram_tensor("x_bf", [P, KD, NTOK], dt.bfloat16, kind="ExternalInput")
    x_f = nc.dram_tensor("x_f", [P, KD, NTOK], dt.float32, kind="ExternalInput")
    gate = nc.dram_tensor("gate", [P, KD, E_], dt.float32, kind="ExternalInput")
    w1t = nc.dram_tensor("w1t", [KF, P, KD, P], dt.bfloat16, kind="ExternalInput")
    w3t = nc.dram_tensor("w3t", [KF, P, KD, P], dt.bfloat16, kind="ExternalInput")
    w2t = nc.dram_tensor("w2t", [MD, P, KF, P], dt.bfloat16, kind="ExternalInput")
    # A1/A3 packed even/odd with 32-aligned expert slots:
    # a1p[:, par, 32*j : 32*j+16] = A1[2*j+par].T  (zeros elsewhere)
    a1p = nc.dram_tensor("a1p", [P, KD, 2, P], dt.bfloat16, kind="ExternalInput")
    a3p = nc.dram_tensor("a3p", [P, KD, 2, P], dt.bfloat16, kind="ExternalInput")
    # B1/B3 packed even/odd with 32-aligned expert slots (rows 32j..32j+16 of
    # plane par hold B[2j+par].T), matching the T-projection psum layout so
    # pairs of delta-matmuls can row-pack via tile_position.
    b1s = nc.dram_tensor("b1s", [KF, P, 2, P], dt.bfloat16, kind="ExternalInput")
    b3s = nc.dram_tensor("b3s", [KF, P, 2, P], dt.bfloat16, kind="ExternalInput")
    a2s = nc.dram_tensor("a2s", [KF, P, ER], dt.bfloat16, kind="ExternalInput")
    b2s = nc.dram_tensor("b2s", [R_, E_, D_], dt.bfloat16, kind="ExternalInput")
    out_t = nc.dram_tensor("out_t", [D_, NTOK], dt.float32, kind="ExternalOutput")

    with TileContext(nc) as tc:
        with (
            tc.tile_pool(name="persist", bufs=1) as persist,
            tc.tile_pool(name="psum_cw", bufs=1, space="PSUM") as ppool_cw,
            tc.tile_pool(name="psum_d", bufs=2, space="PSUM") as ppool_d,
            tc.tile_pool(name="psum_u", bufs=1, space="PSUM") as ppool_u,
            tc.tile_pool(name="dram", bufs=1, space="DRAM") as dpool,
        ):
            for _rep in range(repeat):
                # ---------- persistent tiles (created upfront so the persist
                # pool's footprint is settled before scoped pools stack above) ----
                xbf = persist.tile([P, KD, NTOK], dt.bfloat16)
                nc.sync.dma_start(out=xbf, in_=x_bf[:, :, :])
                ident_f = persist.tile([P, P], dt.float32)
                make_identity(nc, ident_f)
                # H accumulator (bf16) for the whole dff range
                h_big = persist.tile([P, KF, NTOK], dt.bfloat16)
                ewT_sb = persist.tile([E_, NTOK], dt.bfloat16)
                ew_b = []
                for e in range(E_):
                    ewb_t = persist.tile([P, NTOK], dt.bfloat16, tag=f"ewb{e}")
                    ew_b.append(ewb_t)
                t1p, t3p = [None, None], [None, None]
                for par in range(2):
                    t1_t = persist.tile([P, NTOK], dt.bfloat16, tag=f"t1_{par}")
                    t1p[par] = t1_t
                    t3_t = persist.tile([P, NTOK], dt.bfloat16, tag=f"t3_{par}")
                    t3p[par] = t3_t
                uw = []
                for e in range(E_):
                    uw_t = persist.tile([R_, NTOK], dt.bfloat16, tag=f"uw{e}")
                    uw.append(uw_t)

                # ---------- phase 0: routing + lora-down projections ----------
                ew_td = dpool.tile([E_, NTOK], dt.bfloat16)
                p0_cm = tc.tile_pool(name="p0", bufs=3)
                p0 = p0_cm.__enter__()
                xf = p0.tile([P, KD, NTOK], dt.float32, bufs=1)
                nc.sync.dma_start(out=xf, in_=x_f[:, :, :])
                gsb = p0.tile([P, KD, E_], dt.float32, bufs=1)
                nc.sync.dma_start(out=gsb, in_=gate[:, :, :])
                a1sb = p0.tile([P, KD, 2, P], dt.bfloat16, bufs=1)
                nc.sync.dma_start(out=a1sb, in_=a1p[:, :, :, :])
                a3sb = p0.tile([P, KD, 2, P], dt.bfloat16, bufs=1)
                nc.sync.dma_start(out=a3sb, in_=a3p[:, :, :, :])
                ew_pool = p0
                for tt in range(TT):
                    lg = ppool_d.tile([P, E_], dt.float32, tag="d1")
                    for k in range(KD):
                        nc.tensor.matmul(
                            lg,
                            lhsT=xf[:, k, tt * P:(tt + 1) * P],
                            rhs=gsb[:, k, :],
                            start=(k == 0),
                            stop=(k == KD - 1),
                        )
                    l_sb = ew_pool.tile([P, E_], dt.float32, tag="lsb")
                    nc.vector.tensor_copy(l_sb, lg)
                    m1 = ew_pool.tile([P, 1], dt.float32, tag="m1")
                    nc.vector.reduce_max(m1, l_sb, axis=mybir.AxisListType.X)
                    nm1 = ew_pool.tile([P, 1], dt.float32, tag="nm1")
                    nc.vector.tensor_scalar_mul(nm1, m1, -1.0)
                    mask1 = ew_pool.tile([P, E_], dt.float32, tag="mask1")
                    nc.vector.tensor_scalar(
                        mask1, l_sb, scalar1=m1, scalar2=None, op0=op.is_equal
                    )
                    l2 = ew_pool.tile([P, E_], dt.float32, tag="l2")
                    # l2 = mask1 * (-1e30) + l
                    nc.vector.scalar_tensor_tensor(
                        l2, in0=mask1, scalar=-1e30, in1=l_sb, op0=op.mult, op1=op.add
                    )
                    m2 = ew_pool.tile([P, 1], dt.float32, tag="m2")
                    nc.vector.reduce_max(m2, l2, axis=mybir.AxisListType.X)
                    mask2 = ew_pool.tile([P, E_], dt.float32, tag="mask2")
                    nc.vector.tensor_scalar(
                        mask2, l2, scalar1=m2, scalar2=None, op0=op.is_equal
                    )
                    mask = ew_pool.tile([P, E_], dt.float32, tag="mask")
                    nc.vector.tensor_tensor(mask, mask1, mask2, op=op.add)
                    pexp = ew_pool.tile([P, E_], dt.float32, tag="pexp")
                    nc.scalar.activation(pexp, l_sb, AF.Exp, bias=nm1, scale=1.0)
                    pm = ew_pool.tile([P, E_], dt.float32, tag="pm")
                    nc.vector.tensor_tensor(pm, pexp, mask, op=op.mult)
                    den = ew_pool.tile([P, 1], dt.float32, tag="den")
                    nc.vector.reduce_sum(den, pm, axis=mybir.AxisListType.X)
                    rec = ew_pool.tile([P, 1], dt.float32, tag="rec")
                    nc.vector.reciprocal(rec, den)
                    ewt = ew_pool.tile([P, E_], dt.float32, tag="ewt")
                    nc.vector.tensor_scalar_mul(ewt, pm, rec)
                    # transpose [P, E] -> [E, P] and collect into ewT
                    ewtp = ppool_d.tile([E_, P], dt.float32, tag="d3")
                    nc.tensor.transpose(ewtp, ewt, ident_f)
                    nc.scalar.copy(ewT_sb[:, tt * P:(tt + 1) * P], ewtp)

                nc.sync.dma_start(out=ew_td, in_=ewT_sb)
                # broadcast ew rows across partitions: EW_b[e] [P, NTOK]
                for e in range(E_):
                    src = bass.AP(
                        tensor=ew_td.tensor,
                        offset=ew_td.offset + e * NTOK,
                        ap=[[0, P], [1, NTOK]],
                    )
                    nc.sync.dma_start(out=ew_b[e], in_=src)

                # ---------- T1/T3 = stacked per-expert lora-down projections ----------
                # expert 2*j+par sits at rows 32*j..32*j+16 of the `par` chain
                for asb, tlist in ((a1sb, t1p), (a3sb, t3p)):
                    for par in range(2):
                        tp = ppool_d.tile([P, NTOK], dt.float32, tag="d1")
                        for k in range(KD):
                            nc.tensor.matmul(
                                tp,
                                lhsT=asb[:, k, par, :],
                                rhs=xbf[:, k, :],
                                start=(k == 0),
                                stop=(k == KD - 1),
                            )
                        nc.scalar.copy(tlist[par], tp)

                p0_cm.__exit__(None, None, None)
                stream_cm = tc.tile_pool(name="stream", bufs=2)
                stream = stream_cm.__enter__()

                # ---------- U accumulators (per-expert lora-up of h, unscaled) ----------
                u_ps_a = ppool_u.tile([P, NTOK], dt.float32, tag="ua")
                u_ps_b = ppool_u.tile([P, NTOK], dt.float32, tag="ub")
                u_ps = [u_ps_a, u_ps_b]

                # ---------- main dff loop ----------
                ttc = 0  # round-robin counter for p/q engine assignment
                for kt in range(KF):
                    # CW1/CW3 for this dff tile
                    w1sl = stream.tile([P, KD, P], dt.bfloat16, tag="w1sl")
                    nc.sync.dma_start(out=w1sl, in_=w1t[kt, :, :, :])
                    w3sl = stream.tile([P, KD, P], dt.bfloat16, tag="w3sl")
                    nc.sync.dma_start(out=w3sl, in_=w3t[kt, :, :, :])
                    cw1p = ppool_cw.tile([P, NTOK], dt.float32, tag="cw1")
                    cw3p = ppool_cw.tile([P, NTOK], dt.float32, tag="cw3")
                    for k in range(KD):
                        nc.tensor.matmul(
                            cw1p, lhsT=w1sl[:, k, :], rhs=xbf[:, k, :],
                            start=(k == 0), stop=(k == KD - 1),
                        )
                    for k in range(KD):
                        nc.tensor.matmul(
                            cw3p, lhsT=w3sl[:, k, :], rhs=xbf[:, k, :],
                            start=(k == 0), stop=(k == KD - 1),
                        )
                    cw1 = stream.tile([P, NTOK], dt.bfloat16, tag="cw1s")
                    nc.scalar.copy(cw1, cw1p)
                    cw3 = stream.tile([P, NTOK], dt.bfloat16, tag="cw3s")
                    nc.scalar.copy(cw3, cw3p)

                    # per-kt lora weights (even/odd packed, rows 32j hold B[2j+par].T)
                    b1kt = stream.tile([P, 2, P], dt.bfloat16, tag="b1kt")
                    nc.sync.dma_start(out=b1kt, in_=b1s[kt, :, :, :])
                    b3kt = stream.tile([P, 2, P], dt.bfloat16, tag="b3kt")
                    nc.sync.dma_start(out=b3kt, in_=b3s[kt, :, :, :])
                    a2kt = stream.tile([P, ER], dt.bfloat16, tag="a2kt")
                    nc.sync.dma_start(out=a2kt, in_=a2s[kt, :, :])

                    hslice = h_big[:, kt, :]
                    # pairs share a parity and differ in 32-row group, so the two
                    # K=16 delta matmuls of a pair row-pack on the PE
                    for e0, e1 in ((0, 2), (1, 3), (4, 6), (5, 7)):
                        dd = {}
                        for e in (e0, e1):
                            par, j = e % 2, e // 2
                            r0 = 32 * j
                            d1p = ppool_d.tile([P, NTOK], dt.float32, tag="d1")
                            nc.tensor.matmul(
                                d1p, lhsT=b1kt[r0:r0 + R_, par, :],
                                rhs=t1p[par][r0:r0 + R_, :],
                                start=True, stop=True, tile_position=(r0, 0),
                            )
                            d3p = ppool_d.tile([P, NTOK], dt.float32, tag="d3")
                            nc.tensor.matmul(
                                d3p, lhsT=b3kt[r0:r0 + R_, par, :],
                                rhs=t3p[par][r0:r0 + R_, :],
                                start=True, stop=True, tile_position=(r0, 0),
                            )
                            dd[e] = (d1p, d3p)
                        for e in (e0, e1):
                            d1p, d3p = dd[e]
                            j = e // 2
                            w1e = stream.tile([P, NTOK], dt.bfloat16, tag="w1e", bufs=3)
                            nc.vector.tensor_tensor(w1e, cw1, d1p, op=op.add)
                            s_e = stream.tile([P, NTOK], dt.bfloat16, tag="s_e", bufs=3)
                            nc.scalar.activation(s_e, w1e, AF.Silu)
                            w3e = stream.tile([P, NTOK], dt.bfloat16, tag="w3e", bufs=3)
                            nc.vector.tensor_tensor(w3e, cw3, d3p, op=op.add)
                            p_e = stream.tile([P, NTOK], dt.bfloat16, tag="p_e", bufs=3)
                            eng = nc.vector if ttc % 4 == 0 else nc.gpsimd
                            ttc += 1
                            eng.tensor_tensor(p_e, s_e, w3e, op=op.mult)
                            # q = ew * p ; first expert writes H directly, rest add
                            if e == 0:
                                eng = nc.gpsimd
                                eng.tensor_tensor(hslice, p_e, ew_b[e], op=op.mult)
                            else:
                                q_e = stream.tile(
                                    [P, NTOK], dt.bfloat16, tag="q_e", bufs=3
                                )
                                eng = nc.vector if ttc % 4 == 0 else nc.gpsimd
                                ttc += 1
                                eng.tensor_tensor(q_e, p_e, ew_b[e], op=op.mult)
                                nc.vector.tensor_tensor(hslice, hslice, q_e, op=op.add)
                            # U[e] += A2e.T-contraction of (unscaled) p_e
                            nc.tensor.matmul(
                                u_ps[e % 2][32 * j:32 * j + R_, :],
                                lhsT=a2kt[:, e * R_:(e + 1) * R_],
                                rhs=p_e,
                                start=(kt == 0),
                                stop=(kt == KF - 1),
                                tile_position=(0, 32 * j),
                            )

                # ---------- Uw: apply ew column scaling to U ----------
                for e in range(E_):
                    j = e // 2
                    nc.vector.tensor_tensor(
                        uw[e], u_ps[e % 2][32 * j:32 * j + R_, :], ew_b[e][0:R_, :],
                        op=op.mult,
                    )

                # ---------- output GEMM: out = W2 @ H + sum_e B2e @ Uw_e ----------
                KH = KF // 2
                for m in range(MD):
                    outp = ppool_d.tile([P, NTOK], dt.float32, tag="d1")
                    for h in range(2):
                        w2m = stream.tile([P, KH, P], dt.bfloat16, tag="w2m")
                        nc.sync.dma_start(
                            out=w2m, in_=w2t[m, :, h * KH:(h + 1) * KH, :]
                        )
                        for kk in range(KH):
                            kt = h * KH + kk
                            nc.tensor.matmul(
                                outp, lhsT=w2m[:, kk, :], rhs=h_big[:, kt, :],
                                start=(kt == 0), stop=False,
                            )
                    b2m = stream.tile([R_, E_, P], dt.bfloat16, tag="b2m")
                    nc.sync.dma_start(out=b2m, in_=b2s[:, :, m * P:(m + 1) * P])
                    for e in range(E_):
                        nc.tensor.matmul(
                            outp, lhsT=b2m[:, e, :], rhs=uw[e],
                            start=False, stop=(e == E_ - 1),
                        )
                    osb = stream.tile([P, NTOK], dt.float32, tag="osb")
                    nc.scalar.copy(osb, outp)
                    nc.sync.dma_start(out=out_t[m * P:(m + 1) * P, :], in_=osb)

                stream_cm.__exit__(None, None, None)

    nc.compile()
    return nc


def _sw_d(arr):
    """[D, ...] -> [P, KD, ...] partition-major swizzle (d = k*128 + p)."""
    D_ = arr.shape[0]
    rest = arr.shape[1:]
    return np.ascontiguousarray(
        arr.reshape(D_ // 128, 128, *rest).swapaxes(0, 1)
    )


def _pack_a_evenodd(A):
    """A [E, R, D] -> [P, KD, 2, 128] with A[2j+par].T at [:, :, par, 32j:+16]."""
    E_, R_, D_ = A.shape
    out = np.zeros((D_, 2, 128), A.dtype)
    for e in range(E_):
        par, j = e % 2, e // 2
        out[:, par, 32 * j:32 * j + R_] = A[e].T
    return _sw_d(out)


def _pack_b_evenodd(B):
    """B [E, F, R] -> [KF, 128, 2, 128]: B[2j+par].T kt-tiles at
    [kt, 32j:32j+16, par, :]."""
    E_, F_, R_ = B.shape
    out = np.zeros((128, 2, F_), B.dtype)
    for e in range(E_):
        par, j = e % 2, e // 2
        out[32 * j:32 * j + R_, par, :] = B[e].T
    # [row, par, (kt n)] -> [kt, row, par, n]
    return np.ascontiguousarray(
        out.reshape(128, 2, F_ // 128, 128).transpose(2, 0, 1, 3)
    )


def _pack_w_ktiles(WT):
    """WT [K, M] (contraction-major) -> [MT, P, KT, P] where
    out[mt, p, kt, n] = WT[kt*128+p, mt*128+n] — per-(mt) slab is
    partition-major with [KT, 128] contiguous per partition."""
    K_, M_ = WT.shape
    return np.ascontiguousarray(
        WT.reshape(K_ // 128, 128, M_ // 128, 128).transpose(2, 1, 0, 3)
    )


def _prep_inputs(x, W1, W3, W2, gate_w, A1, B1, A3, B3, A2, B2):
    """Host-side packing: transposes + casts, shared across cores."""
    import ml_dtypes

    bf16 = ml_dtypes.bfloat16
    f32 = np.float32

    xT = np.ascontiguousarray(np.asarray(x, f32).T)            # [D, N]
    dff = W1.shape[0]
    shared = {
        "gate": _sw_d(np.ascontiguousarray(np.asarray(gate_w, f32).T)),
        "w1t": _pack_w_ktiles(np.asarray(W1, f32).T.astype(bf16)),
        "w3t": _pack_w_ktiles(np.asarray(W3, f32).T.astype(bf16)),
        "w2t": _pack_w_ktiles(np.asarray(W2, f32).T.astype(bf16)),
        "a1p": _pack_a_evenodd(np.asarray(A1, f32)).astype(bf16),
        "a3p": _pack_a_evenodd(np.asarray(A3, f32)).astype(bf16),
        "b1s": _pack_b_evenodd(np.asarray(B1, f32)).astype(bf16),
        "b3s": _pack_b_evenodd(np.asarray(B3, f32)).astype(bf16),
        "a2s": np.ascontiguousarray(
            np.asarray(A2, f32).transpose(2, 0, 1).reshape(dff // 128, 128, -1)
        ).astype(bf16),
        "b2s": np.ascontiguousarray(np.asarray(B2, f32).transpose(2, 0, 1)).astype(bf16),
    }
    ntok = xT.shape[1] // NCORES
    in_maps = []
    for c in range(NCORES):
        sl = np.ascontiguousarray(xT[:, c * ntok:(c + 1) * ntok])
        m = dict(shared)
        m["x_f"] = _sw_d(sl)
        m["x_bf"] = _sw_d(sl.astype(bf16))
        in_maps.append(m)
    return in_maps


def _ensure_compiled():
    if "exec" not in _CACHE:
        _CACHE["exec"] = _make_exec(build_bass())
    return _CACHE["exec"]


def _make_exec(nc):
    """Build a jitted 8-core shard_map executor for a Bass program.

    Mirrors concourse.bass2jax.run_bass_via_pjrt, but caches the jitted
    callable and keeps real inputs un-donated so device buffers can be
    reused across calls (for timing)."""
    import jax
    import concourse.mybir as mybir
    from concourse import bass2jax
    from jax.experimental.shard_map import shard_map
    from jax.sharding import Mesh, PartitionSpec

    bass2jax.install_neuronx_cc_hook()

    partition_name = (
        nc.partition_id_tensor.name if nc.partition_id_tensor else None
    )
    in_names, out_names, out_avals, zero_outs = [], [], [], []
    for alloc in nc.m.functions[0].allocations:
        if not isinstance(alloc, mybir.MemoryLocationSet):
            continue
        name = alloc.memorylocations[0].name
        if alloc.kind == "ExternalInput":
            if name != partition_name:
                in_names.append(name)
        elif alloc.kind == "ExternalOutput":
            np_dtype = mybir.dt.np(alloc.dtype)
            out_names.append(name)
            out_avals.append(
                jax.core.ShapedArray(tuple(alloc.tensor_shape), np_dtype)
            )
            zero_outs.append(np.zeros(tuple(alloc.tensor_shape), np_dtype))

    n_params = len(in_names)
    n_outs = len(out_names)
    all_names = in_names + out_names
    if partition_name is not None:
        all_names = all_names + [partition_name]

    def _body(*args):
        operands = list(args)
        if partition_name is not None:
            operands.append(bass2jax.partition_id_tensor())
        outs = bass2jax._bass_exec_p.bind(
            *operands,
            out_avals=tuple(out_avals),
            in_names=tuple(all_names),
            out_names=tuple(out_names),
            lowering_input_output_aliases=(),
            sim_require_finite=True,
            sim_require_nnan=True,
            nc=nc,
        )
        return tuple(outs)

    devices = jax.devices()[:NCORES]
    mesh = Mesh(np.asarray(devices), ("core",))
    in_specs = (PartitionSpec("core"),) * (n_params + n_outs)
    out_specs = (PartitionSpec("core"),) * n_outs
    donate = tuple(range(n_params, n_params + n_outs))
    sharded = jax.jit(
        shard_map(
            _body, mesh=mesh, in_specs=in_specs, out_specs=out_specs,
            check_rep=False,
        ),
        donate_argnums=donate,
        keep_unused=True,
    )
    ctx = {
        "fn": sharded,
        "body": _body,
        "n_operands": n_params + n_outs,
        "in_names": in_names,
        "out_names": out_names,
        "zero_outs": zero_outs,
        "mesh": mesh,
    }
    return ctx


def _concat_inputs(in_maps, in_names):
    return [
        np.concatenate([in_maps[c][nm] for c in range(NCORES)], axis=0)
        for nm in in_names
    ]


def _run(ctx, concat_in):
    zeros = [
        np.zeros((NCORES * z.shape[0], *z.shape[1:]), z.dtype)
        for z in ctx["zero_outs"]
    ]
    return ctx["fn"](*concat_in, *zeros)


def kernel(x, W1, W3, W2, gate_w, A1, B1, A3, B3, A2, B2):
    ctx = _ensure_compiled()
    in_maps = _prep_inputs(x, W1, W3, W2, gate_w, A1, B1, A3, B3, A2, B2)
    concat_in = _concat_inputs(in_maps, ctx["in_names"])
    out_arrs = _run(ctx, concat_in)
    ntok = N // NCORES
    res = np.asarray(out_arrs[ctx["out_names"].index("out_t")])
    res = res.reshape(NCORES, D, ntok)
    out = np.empty((N, D), np.float32)
    for c in range(NCORES):
        out[c * ntok:(c + 1) * ntok, :] = res[c].T
    return out


def time_device(inputs, iters=3, ctx=None):
    """Upload all operands once (no donation), then wall-time jitted runs."""
    import time as _time

    import jax
    from jax.experimental.shard_map import shard_map
    from jax.sharding import NamedSharding, PartitionSpec, Mesh

    if ctx is None:
        ctx = _ensure_compiled()
    if "fn_nodonate" not in ctx:
        ctx["fn_nodonate"] = jax.jit(
            shard_map(
                ctx["body"], mesh=ctx["mesh"],
                in_specs=(PartitionSpec("core"),) * ctx["n_operands"],
                out_specs=(PartitionSpec("core"),) * len(ctx["out_names"]),
                check_rep=False,
            ),
            keep_unused=True,
        )
    fn = ctx["fn_nodonate"]
    in_maps = _prep_inputs(**inputs)
    concat_in = _concat_inputs(in_maps, ctx["in_names"])
    zeros = [
        np.zeros((NCORES * z.shape[0], *z.shape[1:]), z.dtype)
        for z in ctx["zero_outs"]
    ]
    sh = NamedSharding(ctx["mesh"], PartitionSpec("core"))
    dev = [jax.device_put(a, sh) for a in (concat_in + zeros)]
    jax.block_until_ready(fn(*dev))  # warmup/compile
    times = []
    for _ in range(iters):
        t0 = _time.perf_counter()
        jax.block_until_ready(fn(*dev))
        times.append(_time.perf_counter() - t0)
    return min(times)



# revision 4
# speedup vs baseline: 1.4494x; 1.4494x over previous
"""MixFFN MoE-routing kernel for Trainium2 (8 NeuronCores, token-parallel).

Math (per token block):
    logits = x @ gate_w.T ; probs = softmax(logits); top2 -> ew [N, E] (dense, rows sum to 1)
    CW1 = x @ W1.T ; CW3 = x @ W3.T
    per expert e:
        w1_e = CW1 + (x @ A1e.T) @ B1e.T
        w3_e = CW3 + (x @ A3e.T) @ B3e.T
        q_e  = silu(w1_e) * (w3_e * ew_e)        (ew commutes into the product)
    out = (sum_e q_e) @ W2.T + sum_e B2e @ (A2e-contraction of q_e)

Key PE-schedule ideas vs the previous version:
  * the per-expert adds (CW + LoRA-delta) run ON THE PE: an identity
    matmul broadcasts CW into the expert's PSUM bank (start=True) and the
    K=16 LoRA delta accumulates on top (stop=True) — no DVE adds.
  * the 16 delta matmuls per dff tile are packed 4-way with
    tile_position row groups 0/32/64/96 (w1+w3 deltas of 2 experts per
    pack, 4 distinct PSUM banks) so they execute concurrently.
  * ew is folded into w3 (q = silu(w1) * (w3*ew)); the A2-contraction
    reads q so the old per-expert U*ew pass disappears.
  * U accumulation is column-tiled 4-way (two packs of 4 experts).
Engine balance targets PE-bound ~11.5us per dff tile with vector/scalar/
gpsimd at ~7-9us so the PE never stalls (keeps HAM at full clock).

Sharding: token-parallel.  Each of the 8 cores gets N/8 = 512 tokens and a
replicated copy of all weights; outputs are disjoint row blocks (no
collectives).  All layout transposes / dtype casts are done host-side.

On-chip layout: feature-on-partition ("transposed"), activations [feat, tok].
"""

import numpy as np

# problem dims (hardcoded per harness contract)
N, D, DFF, E, KTOP, R = 4096, 2048, 8192, 8, 2, 16
NCORES = 8
P = 128

_CACHE = {}


def build_bass(D_=D, DFF_=DFF, E_=E, R_=R, NTOK=N // NCORES, repeat=1):
    """Build the per-core Bass program (same SPMD program on every core)."""
    import concourse.bass as bass
    import concourse.mybir as mybir
    from concourse import bacc
    from concourse.tile import TileContext
    from concourse.masks import make_identity

    dt = mybir.dt
    op = mybir.AluOpType
    AF = mybir.ActivationFunctionType

    KD = D_ // P      # contraction tiles over D
    KF = DFF_ // P    # dff tiles
    MD = D_ // P      # output d tiles
    TT = NTOK // P    # token tiles
    ER = E_ * R_      # stacked expert-rank dim (=128 at full size)
    NCH = 4           # mixed t1/t3 chains (2 experts per chain)

    nc = bacc.Bacc("TRN2", target_bir_lowering=False, debug=False)

    # ---- DRAM I/O ----
    # all inputs pre-swizzled host-side so every DMA is 128 contiguous
    # descriptors (partition-major tiles), not thousands of 256B chunks
    x_bf = nc.dram_tensor("x_bf", [P, KD, NTOK], dt.bfloat16, kind="ExternalInput")
    x_f = nc.dram_tensor("x_f", [P, KD, NTOK], dt.float32, kind="ExternalInput")
    gate = nc.dram_tensor("gate", [P, KD, E_], dt.float32, kind="ExternalInput")
    w1t = nc.dram_tensor("w1t", [KF, P, KD, P], dt.bfloat16, kind="ExternalInput")
    w3t = nc.dram_tensor("w3t", [KF, P, KD, P], dt.bfloat16, kind="ExternalInput")
    w2t = nc.dram_tensor("w2t", [MD, P, KF, P], dt.bfloat16, kind="ExternalInput")
    # A1/A3 mixed chains: chain c serves experts (2c, 2c+1); column slots
    # 0:16->A1[2c], 32:48->A3[2c], 64:80->A1[2c+1], 96:112->A3[2c+1]
    amix = nc.dram_tensor("amix", [P, KD, NCH, P], dt.bfloat16, kind="ExternalInput")
    # B1/B3 mixed chains matching amix slot layout (rows = rank slots)
    bmix = nc.dram_tensor("bmix", [KF, P, NCH, P], dt.bfloat16, kind="ExternalInput")
    a2s = nc.dram_tensor("a2s", [KF, P, ER], dt.bfloat16, kind="ExternalInput")
    b2s = nc.dram_tensor("b2s", [R_, E_, D_], dt.bfloat16, kind="ExternalInput")
    out_t = nc.dram_tensor("out_t", [D_, NTOK], dt.float32, kind="ExternalOutput")

    with TileContext(nc) as tc:
        with (
            tc.tile_pool(name="persist", bufs=1) as persist,
            # 6 banks: tags w1p/w3p x bufs=3 — CW chains and per-expert
            # (ident+delta) accumulations rotate through the same banks
            tc.tile_pool(name="psum_w", bufs=3, space="PSUM") as ppool_w,
            tc.tile_pool(name="psum_u", bufs=1, space="PSUM") as ppool_u,
            tc.tile_pool(name="dram", bufs=1, space="DRAM") as dpool,
        ):
            for _rep in range(repeat):
                # ---------- persistent tiles ----------
                xbf = persist.tile([P, KD, NTOK], dt.bfloat16)
                nc.sync.dma_start(out=xbf, in_=x_bf[:, :, :])
                ident_f = persist.tile([P, P], dt.float32)
                make_identity(nc, ident_f)
                ident_bf = persist.tile([P, P], dt.bfloat16)
                nc.gpsimd.tensor_copy(ident_bf, ident_f)
                # H accumulator (bf16) for the whole dff range
                h_big = persist.tile([P, KF, NTOK], dt.bfloat16)
                ewT_sb = persist.tile([E_, NTOK], dt.bfloat16)
                ew_b = []
                for e in range(E_):
                    ewb_t = persist.tile([P, NTOK], dt.bfloat16, tag=f"ewb{e}")
                    ew_b.append(ewb_t)
                # mixed t1/t3 chains: chain c rows 0:16=t1[2c], 32:48=t3[2c],
                # 64:80=t1[2c+1], 96:112=t3[2c+1]
                tmix = []
                for c in range(NCH):
                    t_t = persist.tile([P, NTOK], dt.bfloat16, tag=f"tmix{c}")
                    tmix.append(t_t)

                # ---------- phase 0: routing + lora-down projections ----------
                ew_td = dpool.tile([E_, NTOK], dt.bfloat16)
                p0_cm = tc.tile_pool(name="p0", bufs=3)
                p0 = p0_cm.__enter__()
                xf = p0.tile([P, KD, NTOK], dt.float32, bufs=1)
                nc.sync.dma_start(out=xf, in_=x_f[:, :, :])
                gsb = p0.tile([P, KD, E_], dt.float32, bufs=1)
                nc.sync.dma_start(out=gsb, in_=gate[:, :, :])
                amx = p0.tile([P, KD, NCH, P], dt.bfloat16, bufs=1)
                nc.sync.dma_start(out=amx, in_=amix[:, :, :, :])
                ew_pool = p0
                for tt in range(TT):
                    lg = ppool_w.tile([P, E_], dt.float32, tag="w1p")
                    for k in range(KD):
                        nc.tensor.matmul(
                            lg,
                            lhsT=xf[:, k, tt * P:(tt + 1) * P],
                            rhs=gsb[:, k, :],
                            start=(k == 0),
                            stop=(k == KD - 1),
                        )
                    l_sb = ew_pool.tile([P, E_], dt.float32, tag="lsb")
                    nc.vector.tensor_copy(l_sb, lg)
                    m1 = ew_pool.tile([P, 1], dt.float32, tag="m1")
                    nc.vector.reduce_max(m1, l_sb, axis=mybir.AxisListType.X)
                    nm1 = ew_pool.tile([P, 1], dt.float32, tag="nm1")
                    nc.vector.tensor_scalar_mul(nm1, m1, -1.0)
                    mask1 = ew_pool.tile([P, E_], dt.float32, tag="mask1")
                    nc.vector.tensor_scalar(
                        mask1, l_sb, scalar1=m1, scalar2=None, op0=op.is_equal
                    )
                    l2 = ew_pool.tile([P, E_], dt.float32, tag="l2")
                    # l2 = mask1 * (-1e30) + l
                    nc.vector.scalar_tensor_tensor(
                        l2, in0=mask1, scalar=-1e30, in1=l_sb, op0=op.mult, op1=op.add
                    )
                    m2 = ew_pool.tile([P, 1], dt.float32, tag="m2")
                    nc.vector.reduce_max(m2, l2, axis=mybir.AxisListType.X)
                    mask2 = ew_pool.tile([P, E_], dt.float32, tag="mask2")
                    nc.vector.tensor_scalar(
                        mask2, l2, scalar1=m2, scalar2=None, op0=op.is_equal
                    )
                    mask = ew_pool.tile([P, E_], dt.float32, tag="mask")
                    nc.vector.tensor_tensor(mask, mask1, mask2, op=op.add)
                    pexp = ew_pool.tile([P, E_], dt.float32, tag="pexp")
                    nc.scalar.activation(pexp, l_sb, AF.Exp, bias=nm1, scale=1.0)
                    pm = ew_pool.tile([P, E_], dt.float32, tag="pm")
                    nc.vector.tensor_tensor(pm, pexp, mask, op=op.mult)
                    den = ew_pool.tile([P, 1], dt.float32, tag="den")
                    nc.vector.reduce_sum(den, pm, axis=mybir.AxisListType.X)
                    rec = ew_pool.tile([P, 1], dt.float32, tag="rec")
                    nc.vector.reciprocal(rec, den)
                    ewt = ew_pool.tile([P, E_], dt.float32, tag="ewt")
                    nc.vector.tensor_scalar_mul(ewt, pm, rec)
                    # transpose [P, E] -> [E, P] and collect into ewT
                    ewtp = ppool_w.tile([E_, P], dt.float32, tag="w3p")
                    nc.tensor.transpose(ewtp, ewt, ident_f)
                    nc.scalar.copy(ewT_sb[:, tt * P:(tt + 1) * P], ewtp)

                nc.sync.dma_start(out=ew_td, in_=ewT_sb)
                # broadcast ew rows across partitions: EW_b[e] [P, NTOK]
                for e in range(E_):
                    src = bass.AP(
                        tensor=ew_td.tensor,
                        offset=ew_td.offset + e * NTOK,
                        ap=[[0, P], [1, NTOK]],
                    )
                    nc.sync.dma_start(out=ew_b[e], in_=src)

                # ---------- mixed-chain lora-down projections ----------
                for c in range(NCH):
                    tp = ppool_w.tile([P, NTOK], dt.float32, tag="w1p")
                    for k in range(KD):
                        nc.tensor.matmul(
                            tp,
                            lhsT=amx[:, k, c, :],
                            rhs=xbf[:, k, :],
                            start=(k == 0),
                            stop=(k == KD - 1),
                        )
                    nc.scalar.copy(tmix[c], tp)

                p0_cm.__exit__(None, None, None)
                stream_cm = tc.tile_pool(name="stream", bufs=2)
                stream = stream_cm.__enter__()

                # ---------- U accumulators (A2-contraction of scaled q) ----------
                # bank a: experts 0-3 at col strips 32j; bank b: experts 4-7
                u_ps_a = ppool_u.tile([P, NTOK], dt.float32, tag="ua")
                u_ps_b = ppool_u.tile([P, NTOK], dt.float32, tag="ub")
                u_ps = [u_ps_a, u_ps_b]

                # ---------- main dff loop ----------
                ttc = 0  # round-robin counter for vector/gpsimd assignment

                def u_pack(bank, kt_, a2_t, qlist):
                    """Col-tiled 4-way A2-contraction for experts 4*bank..+3."""
                    for j in range(4):
                        e = 4 * bank + j
                        nc.tensor.matmul(
                            u_ps[bank][32 * j:32 * j + R_, :],
                            lhsT=a2_t[:, e * R_:(e + 1) * R_],
                            rhs=qlist[e],
                            start=(kt_ == 0),
                            stop=(kt_ == KF - 1),
                            tile_position=(0, 32 * j),
                        )

                prev_q = None
                prev_a2 = None
                for kt in range(KF):
                    # CW1/CW3 for this dff tile
                    w1sl = stream.tile([P, KD, P], dt.bfloat16, tag="w1sl")
                    nc.sync.dma_start(out=w1sl, in_=w1t[kt, :, :, :])
                    w3sl = stream.tile([P, KD, P], dt.bfloat16, tag="w3sl")
                    nc.sync.dma_start(out=w3sl, in_=w3t[kt, :, :, :])
                    cw1p = ppool_w.tile([P, NTOK], dt.float32, tag="w1p")
                    cw3p = ppool_w.tile([P, NTOK], dt.float32, tag="w3p")
                    for k in range(KD):
                        nc.tensor.matmul(
                            cw1p, lhsT=w1sl[:, k, :], rhs=xbf[:, k, :],
                            start=(k == 0), stop=(k == KD - 1),
                        )
                    for k in range(KD):
                        nc.tensor.matmul(
                            cw3p, lhsT=w3sl[:, k, :], rhs=xbf[:, k, :],
                            start=(k == 0), stop=(k == KD - 1),
                        )
                    # deferred second U pack of the previous dff tile (its q's
                    # are certainly ready; keeps the PE from stalling on them)
                    if prev_q is not None:
                        u_pack(1, kt - 1, prev_a2, prev_q)
                    cw1 = stream.tile([P, NTOK], dt.bfloat16, tag="cw1s")
                    nc.scalar.copy(cw1, cw1p)
                    cw3 = stream.tile([P, NTOK], dt.bfloat16, tag="cw3s")
                    nc.scalar.copy(cw3, cw3p)

                    bmk = stream.tile([P, NCH, P], dt.bfloat16, tag="bmk")
                    nc.sync.dma_start(out=bmk, in_=bmix[kt, :, :, :])
                    a2kt = stream.tile([P, ER], dt.bfloat16, tag="a2kt")
                    nc.sync.dma_start(out=a2kt, in_=a2s[kt, :, :])

                    hslice = h_big[:, kt, :]
                    q_of = {}
                    for c in range(NCH):
                        eA, eB = 2 * c, 2 * c + 1
                        # psum banks (tag rotation: 2 experts + cw in flight)
                        w1pA = ppool_w.tile([P, NTOK], dt.float32, tag="w1p")
                        w3pA = ppool_w.tile([P, NTOK], dt.float32, tag="w3p")
                        w1pB = ppool_w.tile([P, NTOK], dt.float32, tag="w1p")
                        w3pB = ppool_w.tile([P, NTOK], dt.float32, tag="w3p")
                        # identity-broadcast CW into the four banks (full-array MMs)
                        nc.tensor.matmul(
                            w1pA, lhsT=ident_bf, rhs=cw1, start=True, stop=False)
                        nc.tensor.matmul(
                            w3pA, lhsT=ident_bf, rhs=cw3, start=True, stop=False)
                        nc.tensor.matmul(
                            w1pB, lhsT=ident_bf, rhs=cw1, start=True, stop=False)
                        nc.tensor.matmul(
                            w3pB, lhsT=ident_bf, rhs=cw3, start=True, stop=False)
                        # 4-way row-tiled delta pack (concurrent K=16 tiles)
                        for slot, tgt in enumerate((w1pA, w3pA, w1pB, w3pB)):
                            r0 = 32 * slot
                            nc.tensor.matmul(
                                tgt,
                                lhsT=bmk[r0:r0 + R_, c, :],
                                rhs=tmix[c][r0:r0 + R_, :],
                                start=False, stop=True,
                                tile_position=(r0, 0),
                            )
                        # first U pack once experts 0-3 all have q (after c==2's
                        # MMs there has been a full chain of PE time for DVE)
                        if c == 3:
                            u_pack(0, kt, a2kt, q_of)
                        # DVE chain per expert
                        for ee, w1p, w3p in ((eA, w1pA, w3pA), (eB, w1pB, w3pB)):
                            s_e = stream.tile([P, NTOK], dt.bfloat16, tag="s_e", bufs=3)
                            nc.scalar.activation(s_e, w1p, AF.Silu)
                            w3q = stream.tile([P, NTOK], dt.bfloat16, tag="w3q", bufs=3)
                            nc.vector.tensor_tensor(w3q, w3p, ew_b[ee], op=op.mult)
                            q_e = stream.tile(
                                [P, NTOK], dt.bfloat16, tag="q_e", bufs=10)
                            if ee == 0:
                                nc.vector.tensor_tensor(q_e, s_e, w3q, op=op.mult)
                            elif ee == 1:
                                nc.vector.tensor_tensor(q_e, s_e, w3q, op=op.mult)
                                nc.gpsimd.tensor_tensor(
                                    hslice, q_of[0], q_e, op=op.add)
                            else:
                                eng = nc.gpsimd if ttc % 2 == 0 else nc.vector
                                ttc += 1
                                eng.tensor_tensor(q_e, s_e, w3q, op=op.mult)
                                eng2 = nc.gpsimd if ttc % 2 == 0 else nc.vector
                                ttc += 1
                                eng2.tensor_tensor(hslice, hslice, q_e, op=op.add)
                            q_of[ee] = q_e
                    prev_q = q_of
                    prev_a2 = a2kt
                u_pack(1, KF - 1, prev_a2, prev_q)

                # ---------- export U (already ew-scaled via q) ----------
                uq = []
                for e in range(E_):
                    bank, j = e // 4, e % 4
                    uq_t = stream.tile([R_, NTOK], dt.bfloat16, tag=f"uq{e}", bufs=1)
                    nc.vector.tensor_copy(uq_t, u_ps[bank][32 * j:32 * j + R_, :])
                    uq.append(uq_t)

                # ---------- output GEMM: out = W2 @ H + sum_e B2e @ Uq_e ----------
                KH = KF // 2
                for m in range(MD):
                    outp = ppool_w.tile([P, NTOK], dt.float32, tag="w1p")
                    for h in range(2):
                        w2m = stream.tile([P, KH, P], dt.bfloat16, tag="w2m")
                        nc.sync.dma_start(
                            out=w2m, in_=w2t[m, :, h * KH:(h + 1) * KH, :]
                        )
                        for kk in range(KH):
                            kt = h * KH + kk
                            nc.tensor.matmul(
                                outp, lhsT=w2m[:, kk, :], rhs=h_big[:, kt, :],
                                start=(kt == 0), stop=False,
                            )
                    b2m = stream.tile([R_, E_, P], dt.bfloat16, tag="b2m")
                    nc.sync.dma_start(out=b2m, in_=b2s[:, :, m * P:(m + 1) * P])
                    for e in range(E_):
                        nc.tensor.matmul(
                            outp, lhsT=b2m[:, e, :], rhs=uq[e],
                            start=False, stop=(e == E_ - 1),
                        )
                    osb = stream.tile([P, NTOK], dt.float32, tag="osb")
                    nc.scalar.copy(osb, outp)
                    nc.sync.dma_start(out=out_t[m * P:(m + 1) * P, :], in_=osb)

                stream_cm.__exit__(None, None, None)

    nc.compile()
    return nc


def _sw_d(arr):
    """[D, ...] -> [P, KD, ...] partition-major swizzle (d = k*128 + p)."""
    D_ = arr.shape[0]
    rest = arr.shape[1:]
    return np.ascontiguousarray(
        arr.reshape(D_ // 128, 128, *rest).swapaxes(0, 1)
    )


def _pack_amix(A1, A3):
    """A1/A3 [E, R, D] -> [P, KD, 4, 128] mixed chains.

    chain c slots: cols 0:16 -> A1[2c].T, 32:48 -> A3[2c].T,
    64:80 -> A1[2c+1].T, 96:112 -> A3[2c+1].T (zeros elsewhere)."""
    E_, R_, D_ = A1.shape
    out = np.zeros((D_, 4, 128), A1.dtype)
    for c in range(4):
        out[:, c, 0:R_] = A1[2 * c].T
        out[:, c, 32:32 + R_] = A3[2 * c].T
        out[:, c, 64:64 + R_] = A1[2 * c + 1].T
        out[:, c, 96:96 + R_] = A3[2 * c + 1].T
    return _sw_d(out)


def _pack_bmix(B1, B3):
    """B1/B3 [E, F, R] -> [KF, 128, 4, 128] mixed chains.

    bmix[kt, r, c, m]: rows 0:16 -> B1[2c].T kt-block, 32:48 -> B3[2c].T,
    64:80 -> B1[2c+1].T, 96:112 -> B3[2c+1].T."""
    E_, F_, R_ = B1.shape
    out = np.zeros((128, 4, F_), B1.dtype)
    for c in range(4):
        out[0:R_, c, :] = B1[2 * c].T
        out[32:32 + R_, c, :] = B3[2 * c].T
        out[64:64 + R_, c, :] = B1[2 * c + 1].T
        out[96:96 + R_, c, :] = B3[2 * c + 1].T
    # [row, c, (kt n)] -> [kt, row, c, n]
    return np.ascontiguousarray(
        out.reshape(128, 4, F_ // 128, 128).transpose(2, 0, 1, 3)
    )


def _pack_w_ktiles(WT):
    """WT [K, M] (contraction-major) -> [MT, P, KT, P] where
    out[mt, p, kt, n] = WT[kt*128+p, mt*128+n] — per-(mt) slab is
    partition-major with [KT, 128] contiguous per partition."""
    K_, M_ = WT.shape
    return np.ascontiguousarray(
        WT.reshape(K_ // 128, 128, M_ // 128, 128).transpose(2, 1, 0, 3)
    )


def _prep_inputs(x, W1, W3, W2, gate_w, A1, B1, A3, B3, A2, B2):
    """Host-side packing: transposes + casts, shared across cores."""
    import ml_dtypes

    bf16 = ml_dtypes.bfloat16
    f32 = np.float32

    xT = np.ascontiguousarray(np.asarray(x, f32).T)            # [D, N]
    dff = W1.shape[0]
    shared = {
        "gate": _sw_d(np.ascontiguousarray(np.asarray(gate_w, f32).T)),
        "w1t": _pack_w_ktiles(np.asarray(W1, f32).T.astype(bf16)),
        "w3t": _pack_w_ktiles(np.asarray(W3, f32).T.astype(bf16)),
        "w2t": _pack_w_ktiles(np.asarray(W2, f32).T.astype(bf16)),
        "amix": _pack_amix(np.asarray(A1, f32), np.asarray(A3, f32)).astype(bf16),
        "bmix": _pack_bmix(np.asarray(B1, f32), np.asarray(B3, f32)).astype(bf16),
        "a2s": np.ascontiguousarray(
            np.asarray(A2, f32).transpose(2, 0, 1).reshape(dff // 128, 128, -1)
        ).astype(bf16),
        "b2s": np.ascontiguousarray(np.asarray(B2, f32).transpose(2, 0, 1)).astype(bf16),
    }
    ntok = xT.shape[1] // NCORES
    in_maps = []
    for c in range(NCORES):
        sl = np.ascontiguousarray(xT[:, c * ntok:(c + 1) * ntok])
        m = dict(shared)
        m["x_f"] = _sw_d(sl)
        m["x_bf"] = _sw_d(sl.astype(bf16))
        in_maps.append(m)
    return in_maps


def _ensure_compiled():
    if "exec" not in _CACHE:
        _CACHE["exec"] = _make_exec(build_bass())
    return _CACHE["exec"]


def _make_exec(nc):
    """Build a jitted 8-core shard_map executor for a Bass program.

    Mirrors concourse.bass2jax.run_bass_via_pjrt, but caches the jitted
    callable and keeps real inputs un-donated so device buffers can be
    reused across calls (for timing)."""
    import jax
    import concourse.mybir as mybir
    from concourse import bass2jax
    from jax.experimental.shard_map import shard_map
    from jax.sharding import Mesh, PartitionSpec

    bass2jax.install_neuronx_cc_hook()

    partition_name = (
        nc.partition_id_tensor.name if nc.partition_id_tensor else None
    )
    in_names, out_names, out_avals, zero_outs = [], [], [], []
    for alloc in nc.m.functions[0].allocations:
        if not isinstance(alloc, mybir.MemoryLocationSet):
            continue
        name = alloc.memorylocations[0].name
        if alloc.kind == "ExternalInput":
            if name != partition_name:
                in_names.append(name)
        elif alloc.kind == "ExternalOutput":
            np_dtype = mybir.dt.np(alloc.dtype)
            out_names.append(name)
            out_avals.append(
                jax.core.ShapedArray(tuple(alloc.tensor_shape), np_dtype)
            )
            zero_outs.append(np.zeros(tuple(alloc.tensor_shape), np_dtype))

    n_params = len(in_names)
    n_outs = len(out_names)
    all_names = in_names + out_names
    if partition_name is not None:
        all_names = all_names + [partition_name]

    def _body(*args):
        operands = list(args)
        if partition_name is not None:
            operands.append(bass2jax.partition_id_tensor())
        outs = bass2jax._bass_exec_p.bind(
            *operands,
            out_avals=tuple(out_avals),
            in_names=tuple(all_names),
            out_names=tuple(out_names),
            lowering_input_output_aliases=(),
            sim_require_finite=True,
            sim_require_nnan=True,
            nc=nc,
        )
        return tuple(outs)

    devices = jax.devices()[:NCORES]
    mesh = Mesh(np.asarray(devices), ("core",))
    in_specs = (PartitionSpec("core"),) * (n_params + n_outs)
    out_specs = (PartitionSpec("core"),) * n_outs
    donate = tuple(range(n_params, n_params + n_outs))
    sharded = jax.jit(
        shard_map(
            _body, mesh=mesh, in_specs=in_specs, out_specs=out_specs,
            check_rep=False,
        ),
        donate_argnums=donate,
        keep_unused=True,
    )
    ctx = {
        "fn": sharded,
        "body": _body,
        "n_operands": n_params + n_outs,
        "in_names": in_names,
        "out_names": out_names,
        "zero_outs": zero_outs,
        "mesh": mesh,
    }
    return ctx


def _concat_inputs(in_maps, in_names):
    return [
        np.concatenate([in_maps[c][nm] for c in range(NCORES)], axis=0)
        for nm in in_names
    ]


def _run(ctx, concat_in):
    zeros = [
        np.zeros((NCORES * z.shape[0], *z.shape[1:]), z.dtype)
        for z in ctx["zero_outs"]
    ]
    return ctx["fn"](*concat_in, *zeros)


def kernel(x, W1, W3, W2, gate_w, A1, B1, A3, B3, A2, B2):
    ctx = _ensure_compiled()
    in_maps = _prep_inputs(x, W1, W3, W2, gate_w, A1, B1, A3, B3, A2, B2)
    concat_in = _concat_inputs(in_maps, ctx["in_names"])
    out_arrs = _run(ctx, concat_in)
    ntok = N // NCORES
    res = np.asarray(out_arrs[ctx["out_names"].index("out_t")])
    res = res.reshape(NCORES, D, ntok)
    out = np.empty((N, D), np.float32)
    for c in range(NCORES):
        out[c * ntok:(c + 1) * ntok, :] = res[c].T
    return out


def time_device(inputs, iters=3, ctx=None):
    """Upload all operands once (no donation), then wall-time jitted runs."""
    import time as _time

    import jax
    from jax.experimental.shard_map import shard_map
    from jax.sharding import NamedSharding, PartitionSpec, Mesh

    if ctx is None:
        ctx = _ensure_compiled()
    if "fn_nodonate" not in ctx:
        ctx["fn_nodonate"] = jax.jit(
            shard_map(
                ctx["body"], mesh=ctx["mesh"],
                in_specs=(PartitionSpec("core"),) * ctx["n_operands"],
                out_specs=(PartitionSpec("core"),) * len(ctx["out_names"]),
                check_rep=False,
            ),
            keep_unused=True,
        )
    fn = ctx["fn_nodonate"]
    in_maps = _prep_inputs(**inputs)
    concat_in = _concat_inputs(in_maps, ctx["in_names"])
    zeros = [
        np.zeros((NCORES * z.shape[0], *z.shape[1:]), z.dtype)
        for z in ctx["zero_outs"]
    ]
    sh = NamedSharding(ctx["mesh"], PartitionSpec("core"))
    dev = [jax.device_put(a, sh) for a in (concat_in + zeros)]
    jax.block_until_ready(fn(*dev))  # warmup/compile
    times = []
    for _ in range(iters):
        t0 = _time.perf_counter()
        jax.block_until_ready(fn(*dev))
        times.append(_time.perf_counter() - t0)
    return min(times)
